# revision 10
# baseline (speedup 1.0000x reference)
"""KeyedLSTM Trainium2 kernel.

Tensor-parallel split of the 4H gate dim across 8 cores; each core owns 256
h rows (2 k-tiles) and their 4 gate blocks (8 m-tiles of 128 rows, ordered
[g,i,f,o] per 128-row sub-block). An ncfw AllGather of the fp16 h slices
(18KB per rank) runs every step.

- The key-gate recurrence (KB=4 over KL=16 steps) runs in lockstep with the
  main recurrence as 4 extra batch columns (N=36) through the same weights,
  so the separate key phase (16 extra all-gathered steps) disappears.

- x @ W_ih.T + bias is produced into an SBUF ring (fp16, 48 steps = 3 chunk
  windows) by matmuls interleaved into the step loop (one 512-token m-tile
  every other step), replacing the bulk precompute phase and its xw DRAM
  round-trip; the producer matmuls run inside the AllGather wait.

- The whole x path is fp16 (inputs converted host-side), halving HBM
  traffic; h is exchanged in fp16; c stays fp32.

- m-outer matmul order (0,4,1,5,2,6,3,7) closes the g/i/f PSUM groups
  first so the activation chain overlaps the o-gate matmuls; the
  gathered-h loads are split across the Sync and Activation HWDGE queues.
"""

import math
import os
import sys

import numpy as np

for _p in (
    "/root/.axon_site",
    "/root/.axon_site/_ro/trn_rl_repo",
    "/root/.axon_site/_ro/pypackages",
    "/opt/trn_rl_repo",
):
    if os.path.isdir(_p) and _p not in sys.path:
        sys.path.append(_p)

import concourse.bacc as bacc
import concourse.bass_utils as bass_utils
import concourse.mybir as mybir
import concourse.tile as tile
from concourse.ap import AP
from concourse.tile_rust import add_dep_helper

AF = mybir.ActivationFunctionType
ALU = mybir.AluOpType
DT = mybir.dt

B, S, I, H = 32, 256, 1024, 2048
KB, KL = 4, 16
NCORES = 8
HLOC = H // NCORES  # 256 h rows per core
MT = 8  # m-tiles of 128 gate rows per core
KT_I = I // 128  # 8
KT_H = H // 128  # 16
NT = B + KB  # 36 columns: 32 main batch + 4 key batch
RING = 48  # xw ring depth in steps (3 chunk windows live at once)
CH = 16  # steps per xw producer chunk (512 tokens)

_GOFF = {"i": 0, "f": H, "g": 2 * H, "o": 3 * H}
# m-tile order per 128-row sub-block: [g,i,f,o] so within each sub-block the
# tanh gate is index 0, sigmoid gates are 1:4.
_ORDER = ("g", "i", "f", "o")


def _rows_for_core(j):
    rows = []
    for p in range(2):
        base = j * HLOC + p * 128
        for g in _ORDER:
            o = _GOFF[g] + base
            rows.extend(range(o, o + 128))
    return np.asarray(rows, dtype=np.int64)


def _build_program(s_steps):
    ttok = s_steps * B
    nch = math.ceil(s_steps / CH)

    nc = bacc.Bacc(
        "TRN2",
        target_bir_lowering=False,
        debug=False,
        enable_asserts=True,
        num_devices=NCORES,
    )

    xt = nc.dram_tensor("xt", [I, ttok], DT.float16, kind="ExternalInput").ap()
    kt = nc.dram_tensor("kt", [I, KL * KB], DT.float16, kind="ExternalInput").ap()
    wih = nc.dram_tensor("wih", [I, MT * 128], DT.float16, kind="ExternalInput").ap()
    whh = nc.dram_tensor("whh", [H, MT * 128], DT.float16, kind="ExternalInput").ap()
    bias = nc.dram_tensor("bias", [MT * 128], DT.float32, kind="ExternalInput").ap()
    out = nc.dram_tensor(
        "out", [s_steps, 128, 2, NT], DT.float16, kind="ExternalOutput"
    ).ap()

    deferred = []  # (BassInstruction, sem, value): patched post-Tile
    last_on = {}

    def chain(key, binst):
        prev = last_on.get(key)
        if prev is not None:
            add_dep_helper(binst.ins, prev.ins, False, f"chain-{key}")
        last_on[key] = binst
        return binst

    def c_pe(b):
        return chain("pe", b)

    def c_dve(b):
        return chain("dve", b)

    def c_act(b):
        return chain("act", b)

    def c_pl(b):
        return chain("pl", b)

    with tile.TileContext(nc) as tc:
        rg = [list(range(NCORES))]
        with (
            tc.tile_pool(name="const", bufs=1) as cpool,
            tc.tile_pool(name="xin", bufs=2) as xin_pool,
            tc.tile_pool(name="gps", bufs=2, space="PSUM") as gps_pool,
            tc.tile_pool(name="xwps", bufs=2, space="PSUM") as xwps_pool,
            tc.tile_pool(name="tmp", bufs=2) as tmp_pool,
            tc.tile_pool(name="cdram", bufs=2, space="DRAM") as cdram_pool,
        ):
            whh_sb = cpool.tile([128, KT_H, MT, 128], DT.float16)
            wih_sb = cpool.tile([128, KT_I, MT, 128], DT.float16)
            bias_sb = cpool.tile([128, MT], DT.float32)
            kt_sb = cpool.tile([128, KT_I, KL * KB], DT.float16)
            xr = cpool.tile([128, RING, MT, NT], DT.float16)
            h_sb = cpool.tile([128, KT_H, NT], DT.float16)
            hstage = cpool.tile([128, 2, 2, NT], DT.float16)
            c_sb = cpool.tile([128, 2, NT], DT.float32)

            whh_re = whh.rearrange("(k p) m -> p k m", p=128)
            for g in range(4):
                nc.sync.dma_start(
                    whh_sb[:, 4 * g : 4 * (g + 1), :, :],
                    whh_re[:, 4 * g : 4 * (g + 1), :],
                )
            nc.sync.dma_start(wih_sb[:], wih.rearrange("(k p) m -> p k m", p=128))
            nc.sync.dma_start(bias_sb[:], bias.rearrange("(m p) -> p m", p=128))
            nc.sync.dma_start(kt_sb[:], kt.rearrange("(k p) t -> p k t", p=128))

            c_dve(nc.vector.memset(h_sb[:], 0.0))
            c_dve(nc.vector.memset(c_sb[:], 0.0))
            c_dve(nc.vector.memset(xr[:], 0.0))

            # ---- xw producer helpers ----
            def chunk_dma(c):
                n0 = c * CH * B
                n1 = min(ttok, n0 + CH * B)
                xch = xin_pool.tile([128, KT_I, CH * B], DT.float16, tag="xch")
                nc.sync.dma_start(
                    xch[:, :, : n1 - n0],
                    xt.rearrange("(k p) t -> p k t", p=128)[:, :, n0:n1],
                )
                return xch

            def chunk_mtile(xch, c, m):
                n0 = c * CH * B
                ncols = min(ttok, n0 + CH * B) - n0
                nstep = ncols // B
                pxw = xwps_pool.tile([128, CH * B], DT.float32, tag="pxw")
                for k in range(KT_I):
                    c_pe(
                        nc.tensor.matmul(
                            pxw[:, :ncols],
                            wih_sb[:, k, m, :],
                            xch[:, k, :ncols],
                            start=(k == 0),
                            stop=(k == KT_I - 1),
                        )
                    )
                s0 = (c % 3) * CH
                c_act(
                    nc.scalar.activation(
                        xr[:, s0 : s0 + nstep, m, 0:B],
                        pxw[:, :ncols].rearrange("p (s b) -> p s b", b=B),
                        AF.Identity,
                        bias=bias_sb[:, m : m + 1],
                    )
                )

            # ---- prologue: chunks 0,1 plus key-side xw ----
            for c in range(min(2, nch)):
                xch = chunk_dma(c)
                for m in range(MT):
                    chunk_mtile(xch, c, m)

            pk = xwps_pool.tile([128, MT, KL * KB], DT.float32, tag="pk")
            for m in range(MT):
                for k in range(KT_I):
                    c_pe(
                        nc.tensor.matmul(
                            pk[:, m, :],
                            wih_sb[:, k, m, :],
                            kt_sb[:, k, :],
                            start=(k == 0),
                            stop=(k == KT_I - 1),
                        )
                    )
            for m in range(MT):
                c_act(
                    nc.scalar.activation(
                        xr[:, 0:KL, m, B:NT],
                        pk[:, m, :].rearrange("p (s b) -> p s b", b=KB),
                        AF.Identity,
                        bias=bias_sb[:, m : m + 1],
                    )
                )

            # ---- main loop ----
            xch_cur = None
            for t in range(s_steps):
                p = t % 2
                mp = (t - 1) % 2
                slot = (t // CH % 3) * CH + t % CH

                # producer work for chunk c = t//CH + 2 (2 windows ahead)
                c = t // CH + 2
                w = t % CH
                if c < nch:
                    if w == 0:
                        xch_cur = chunk_dma(c)
                    if w % 2 == 0 and w // 2 < MT:
                        chunk_mtile(xch_cur, c, w // 2)

                # recurrence matmuls (h_sb holds the gathered h of step t-1)
                # m-outer so PSUM groups open/close sequentially; g/i/f gates
                # of both sub-blocks finish first so the chain starts early.
                ps = gps_pool.tile([128, MT, NT], DT.float32, tag="gps")
                for m in (0, 4, 1, 5, 2, 6, 3, 7):
                    for k in range(KT_H):
                        c_pe(
                            nc.tensor.matmul(
                                ps[:, m, :],
                                whh_sb[:, k, m, :],
                                h_sb[:, k, :],
                                start=(k == 0),
                                stop=(k == KT_H - 1),
                            )
                        )

                # activation / cell-update chain
                gpre = tmp_pool.tile([128, MT, NT], DT.float32, tag="gpre")
                c_dve(nc.vector.tensor_add(gpre[:], ps[:], xr[:, slot, :, :]))
                gv = gpre.rearrange("p (s g) c -> p s g c", s=2)
                gact = tmp_pool.tile([128, MT, NT], DT.float32, tag="gact")
                ga = gact.rearrange("p (s g) c -> p s g c", s=2)
                c_act(nc.scalar.activation(ga[:, :, 0, :], gv[:, :, 0, :], AF.Tanh))
                c_act(
                    nc.scalar.activation(
                        ga[:, :, 1:3, :], gv[:, :, 1:3, :], AF.Sigmoid
                    )
                )
                c_act(
                    nc.scalar.activation(ga[:, :, 3, :], gv[:, :, 3, :], AF.Sigmoid)
                )
                t1 = tmp_pool.tile([128, 2, NT], DT.float32, tag="t1")
                t2 = tmp_pool.tile([128, 2, NT], DT.float32, tag="t2")
                c_dve(nc.vector.tensor_mul(t1[:], ga[:, :, 1, :], ga[:, :, 0, :]))
                c_dve(nc.vector.tensor_mul(t2[:], ga[:, :, 2, :], c_sb[:]))
                c_dve(nc.vector.tensor_add(c_sb[:], t1[:], t2[:]))
                th = tmp_pool.tile([128, 2, NT], DT.float32, tag="th")
                c_act(nc.scalar.activation(th[:], c_sb[:], AF.Tanh))

                if t < KL:
                    h16 = tmp_pool.tile([128, 2, NT], DT.float16, tag="h16")
                    c_dve(nc.vector.tensor_mul(h16[:], ga[:, :, 3, :], th[:]))
                    nc.scalar.dma_start(out[t], h16[:])
                    fs = tmp_pool.tile([128, 2], DT.float32, tag="fs")
                    mult = tmp_pool.tile([128, 2], DT.float32, tag="mult")
                    c_dve(
                        nc.vector.tensor_reduce(
                            fs[:], ga[:, :, 2, B:NT], mybir.AxisListType.X, ALU.add
                        )
                    )
                    c_dve(nc.vector.tensor_scalar_mul(mult[:], fs[:], 1.0 / KB))
                    for s_ in range(2):
                        c_dve(
                            nc.vector.tensor_scalar_mul(
                                hstage[:, p, s_, 0:B],
                                h16[:, s_, 0:B],
                                mult[:, s_ : s_ + 1],
                            )
                        )
                        c_dve(
                            nc.vector.tensor_copy(
                                hstage[:, p, s_, B:NT], h16[:, s_, B:NT]
                            )
                        )
                        c_dve(
                            nc.vector.tensor_scalar_mul(
                                c_sb[:, s_, 0:B],
                                c_sb[:, s_, 0:B],
                                mult[:, s_ : s_ + 1],
                            )
                        )
                else:
                    c_dve(
                        nc.vector.tensor_mul(
                            hstage[:, p, :, :], ga[:, :, 3, :], th[:]
                        )
                    )

                # all-gather h via ncfw collective; ag_in first - it is on
                # the critical path, the out DMA is not.
                ag_in = cdram_pool.tile([2 * 128, NT], DT.float16, tag="agin")
                nc.sync.dma_start(
                    ag_in.rearrange("(s p) b -> p s b", p=128), hstage[:, p, :, :]
                )
                if t >= KL:
                    nc.scalar.dma_start(out[t], hstage[:, p, :, :])
                ag_out = cdram_pool.tile(
                    [H, NT], DT.float16, tag="agout", addr_space="Shared"
                )
                c_pl(
                    nc.gpsimd.collective_compute(
                        "AllGather",
                        ALU.bypass,
                        replica_groups=rg,
                        ins=[ag_in.opt()],
                        outs=[ag_out.opt()],
                    )
                )
                ag_re = ag_out.rearrange("(k p) b -> p k b", p=128)
                nc.sync.dma_start(h_sb[:, 0:8, :], ag_re[:, 0:8, :])
                nc.scalar.dma_start(h_sb[:, 8:16, :], ag_re[:, 8:16, :])

    for binst, sem, val in deferred:
        binst.wait_op(sem, val, "sem-ge")

    nc.compile()
    return nc


def _prepare_inputs(x, key_seq, weight_ih, weight_hh, bias_ih, bias_hh, s_steps):
    x = np.ascontiguousarray(np.asarray(x, dtype=np.float32)[:, :s_steps, :])
    key_seq = np.asarray(key_seq, dtype=np.float32)
    weight_ih = np.asarray(weight_ih, dtype=np.float32)
    weight_hh = np.asarray(weight_hh, dtype=np.float32)
    b = np.asarray(bias_ih, dtype=np.float32) + np.asarray(bias_hh, dtype=np.float32)

    xt = np.ascontiguousarray(
        x.transpose(2, 1, 0).reshape(I, s_steps * B).astype(np.float16)
    )
    kt = np.ascontiguousarray(
        key_seq.transpose(2, 1, 0).reshape(I, KL * KB).astype(np.float16)
    )

    in_maps = []
    for j in range(NCORES):
        rows = _rows_for_core(j)
        in_maps.append(
            {
                "xt": xt,
                "kt": kt,
                "wih": np.ascontiguousarray(weight_ih[rows].T.astype(np.float16)),
                "whh": np.ascontiguousarray(weight_hh[rows].T.astype(np.float16)),
                "bias": np.ascontiguousarray(b[rows]),
            }
        )
    return in_maps


_NC_CACHE = {}


def _run(x, key_seq, weight_ih, weight_hh, bias_ih, bias_hh, s_steps, trace=False):
    if s_steps not in _NC_CACHE:
        _NC_CACHE[s_steps] = _build_program(s_steps)
    nc = _NC_CACHE[s_steps]
    in_maps = _prepare_inputs(
        x, key_seq, weight_ih, weight_hh, bias_ih, bias_hh, s_steps
    )
    res = bass_utils.run_bass_kernel_spmd(
        nc, in_maps, core_ids=list(range(NCORES)), trace=trace
    )
    full = np.empty((s_steps, B, H), dtype=np.float32)
    for j in range(NCORES):
        o = np.asarray(res.results[j]["out"], dtype=np.float32)  # [T,128,2,NT]
        # full[t, b, j*256 + s*128 + p] = o[t, p, s, b]
        full[:, :, j * HLOC : (j + 1) * HLOC] = (
            o[:, :, :, :B].transpose(0, 3, 2, 1).reshape(s_steps, B, HLOC)
        )
    return full, res


def kernel(x, key_seq, weight_ih, weight_hh, bias_ih, bias_hh):
    s_steps = int(os.environ.get("KEYED_LSTM_STEPS", S))
    trace = os.environ.get("KEYED_LSTM_TRACE", "0") == "1"
    full, _res = _run(
        x, key_seq, weight_ih, weight_hh, bias_ih, bias_hh, s_steps, trace=trace
    )
    return full


# revision 11
# speedup vs baseline: 1.0966x; 1.0966x over previous
"""KeyedLSTM Trainium2 kernel.

Tensor-parallel split of the 4H gate dim across 8 cores; each core owns 256
h rows (2 k-tiles) and their 4 gate blocks (8 m-tiles of 128 rows, ordered
[g,i,f,o] per 128-row sub-block). An ncfw AllGather of the fp16 h slices
(18KB per rank) runs every step.

- The key-gate recurrence (KB=4 over KL=16 steps) runs in lockstep with the
  main recurrence as 4 extra batch columns (N=36) through the same weights,
  so the separate key phase (16 extra all-gathered steps) disappears.

- x @ W_ih.T + bias is produced into an SBUF ring (fp16, 48 steps = 3 chunk
  windows) by matmuls interleaved into the step loop (one 512-token m-tile
  every other step), replacing the bulk precompute phase and its xw DRAM
  round-trip; the producer matmuls run inside the AllGather wait.

- The whole x path is fp16 (inputs converted host-side), halving HBM
  traffic; h is exchanged in fp16; c stays fp32.

- m-outer matmul order (0,4,1,5,2,6,3,7) closes the g/i/f PSUM groups
  first so the activation chain overlaps the o-gate matmuls; the
  gathered-h loads are split across the Sync and Activation HWDGE queues.
"""

import math
import os
import sys

import numpy as np

for _p in (
    "/root/.axon_site",
    "/root/.axon_site/_ro/trn_rl_repo",
    "/root/.axon_site/_ro/pypackages",
    "/opt/trn_rl_repo",
):
    if os.path.isdir(_p) and _p not in sys.path:
        sys.path.append(_p)

import concourse.bacc as bacc
import concourse.bass_utils as bass_utils
import concourse.mybir as mybir
import concourse.tile as tile
from concourse.ap import AP
from concourse.tile_rust import add_dep_helper

AF = mybir.ActivationFunctionType
ALU = mybir.AluOpType
DT = mybir.dt

B, S, I, H = 32, 256, 1024, 2048
KB, KL = 4, 16
NCORES = 8
HLOC = H // NCORES  # 256 h rows per core
MT = 8  # m-tiles of 128 gate rows per core
KT_I = I // 128  # 8
KT_H = H // 128  # 16
NT = B + KB  # 36 columns: 32 main batch + 4 key batch
RING = 48  # xw ring depth in steps (3 chunk windows live at once)
CH = 16  # steps per xw producer chunk (512 tokens)

_GOFF = {"i": 0, "f": H, "g": 2 * H, "o": 3 * H}
# m-tile order per 128-row sub-block: [g,i,f,o] so within each sub-block the
# tanh gate is index 0, sigmoid gates are 1:4.
_ORDER = ("g", "i", "f", "o")


def _rows_for_core(j):
    rows = []
    for p in range(2):
        base = j * HLOC + p * 128
        for g in _ORDER:
            o = _GOFF[g] + base
            rows.extend(range(o, o + 128))
    return np.asarray(rows, dtype=np.int64)


def _build_program(s_steps):
    ttok = s_steps * B
    nch = math.ceil(s_steps / CH)

    nc = bacc.Bacc(
        "TRN2",
        target_bir_lowering=False,
        debug=False,
        enable_asserts=True,
        num_devices=NCORES,
    )

    xt = nc.dram_tensor("xt", [I, ttok], DT.float16, kind="ExternalInput").ap()
    kt = nc.dram_tensor("kt", [I, KL * KB], DT.float16, kind="ExternalInput").ap()
    wih = nc.dram_tensor("wih", [I, MT * 128], DT.float16, kind="ExternalInput").ap()
    whh = nc.dram_tensor("whh", [H, MT * 128], DT.float16, kind="ExternalInput").ap()
    bias = nc.dram_tensor("bias", [MT * 128], DT.float32, kind="ExternalInput").ap()
    out = nc.dram_tensor(
        "out", [s_steps, 128, 2, NT], DT.float16, kind="ExternalOutput"
    ).ap()

    deferred = []  # (BassInstruction, sem, value): patched post-Tile
    last_on = {}

    def chain(key, binst):
        prev = last_on.get(key)
        if prev is not None:
            add_dep_helper(binst.ins, prev.ins, False, f"chain-{key}")
        last_on[key] = binst
        return binst

    def c_pe(b):
        return chain("pe", b)

    def c_dve(b):
        return chain("dve", b)

    def c_act(b):
        return chain("act", b)

    def c_pl(b):
        return chain("pl", b)

    with tile.TileContext(nc) as tc:
        rg = [list(range(NCORES))]
        with (
            tc.tile_pool(name="const", bufs=1) as cpool,
            tc.tile_pool(name="xin", bufs=2) as xin_pool,
            tc.tile_pool(name="gps", bufs=2, space="PSUM") as gps_pool,
            tc.tile_pool(name="xwps", bufs=2, space="PSUM") as xwps_pool,
            tc.tile_pool(name="tmp", bufs=2) as tmp_pool,
            tc.tile_pool(name="cdram", bufs=2, space="DRAM") as cdram_pool,
        ):
            whh_sb = cpool.tile([128, KT_H, MT, 128], DT.float16)
            wih_sb = cpool.tile([128, KT_I, MT, 128], DT.float16)
            bias_sb = cpool.tile([128, MT], DT.float32)
            kt_sb = cpool.tile([128, KT_I, KL * KB], DT.float16)
            xr = cpool.tile([128, RING, MT, NT], DT.float16)
            h_sb = cpool.tile([128, KT_H, NT], DT.float16)
            hstage = cpool.tile([128, 2, 2, NT], DT.float16)
            c_sb = cpool.tile([128, 2, NT], DT.float32)

            whh_re = whh.rearrange("(k p) m -> p k m", p=128)
            for g in range(4):
                nc.sync.dma_start(
                    whh_sb[:, 4 * g : 4 * (g + 1), :, :],
                    whh_re[:, 4 * g : 4 * (g + 1), :],
                )
            nc.sync.dma_start(wih_sb[:], wih.rearrange("(k p) m -> p k m", p=128))
            nc.sync.dma_start(bias_sb[:], bias.rearrange("(m p) -> p m", p=128))
            nc.sync.dma_start(kt_sb[:], kt.rearrange("(k p) t -> p k t", p=128))

            c_dve(nc.vector.memset(h_sb[:], 0.0))
            c_dve(nc.vector.memset(c_sb[:], 0.0))
            c_dve(nc.vector.memset(xr[:], 0.0))

            # ---- xw producer helpers ----
            def chunk_dma(c):
                n0 = c * CH * B
                n1 = min(ttok, n0 + CH * B)
                xch = xin_pool.tile([128, KT_I, CH * B], DT.float16, tag="xch")
                nc.sync.dma_start(
                    xch[:, :, : n1 - n0],
                    xt.rearrange("(k p) t -> p k t", p=128)[:, :, n0:n1],
                )
                return xch

            def chunk_mtile(xch, c, m):
                n0 = c * CH * B
                ncols = min(ttok, n0 + CH * B) - n0
                nstep = ncols // B
                pxw = xwps_pool.tile([128, CH * B], DT.float32, tag="pxw")
                for k in range(KT_I):
                    c_pe(
                        nc.tensor.matmul(
                            pxw[:, :ncols],
                            wih_sb[:, k, m, :],
                            xch[:, k, :ncols],
                            start=(k == 0),
                            stop=(k == KT_I - 1),
                        )
                    )
                s0 = (c % 3) * CH
                c_act(
                    nc.scalar.activation(
                        xr[:, s0 : s0 + nstep, m, 0:B],
                        pxw[:, :ncols].rearrange("p (s b) -> p s b", b=B),
                        AF.Identity,
                        bias=bias_sb[:, m : m + 1],
                    )
                )

            # ---- prologue: chunks 0,1 plus key-side xw ----
            for c in range(min(2, nch)):
                xch = chunk_dma(c)
                for m in range(MT):
                    chunk_mtile(xch, c, m)

            pk = xwps_pool.tile([128, MT, KL * KB], DT.float32, tag="pk")
            for m in range(MT):
                for k in range(KT_I):
                    c_pe(
                        nc.tensor.matmul(
                            pk[:, m, :],
                            wih_sb[:, k, m, :],
                            kt_sb[:, k, :],
                            start=(k == 0),
                            stop=(k == KT_I - 1),
                        )
                    )
            for m in range(MT):
                c_act(
                    nc.scalar.activation(
                        xr[:, 0:KL, m, B:NT],
                        pk[:, m, :].rearrange("p (s b) -> p s b", b=KB),
                        AF.Identity,
                        bias=bias_sb[:, m : m + 1],
                    )
                )

            # ---- main loop ----
            xch_cur = None
            for t in range(s_steps):
                p = t % 2
                mp = (t - 1) % 2
                slot = (t // CH % 3) * CH + t % CH

                # producer work for chunk c = t//CH + 2 (2 windows ahead)
                c = t // CH + 2
                w = t % CH
                if c < nch:
                    if w == 0:
                        xch_cur = chunk_dma(c)
                    if w % 2 == 0 and w // 2 < MT:
                        chunk_mtile(xch_cur, c, w // 2)

                # recurrence matmuls (h_sb holds the gathered h of step t-1)
                # m-outer so PSUM groups open/close sequentially; g/i/f gates
                # of both sub-blocks finish first so the chain starts early.
                ps = gps_pool.tile([128, MT, NT], DT.float32, tag="gps")
                for m in (0, 4, 1, 5, 2, 6, 3, 7):
                    for k in range(KT_H):
                        c_pe(
                            nc.tensor.matmul(
                                ps[:, m, :],
                                whh_sb[:, k, m, :],
                                h_sb[:, k, :],
                                start=(k == 0),
                                stop=(k == KT_H - 1),
                            )
                        )

                # activation / cell-update chain
                gpre = tmp_pool.tile([128, MT, NT], DT.float32, tag="gpre")
                c_dve(nc.vector.tensor_add(gpre[:], ps[:], xr[:, slot, :, :]))
                gv = gpre.rearrange("p (s g) c -> p s g c", s=2)
                gact = tmp_pool.tile([128, MT, NT], DT.float32, tag="gact")
                ga = gact.rearrange("p (s g) c -> p s g c", s=2)
                c_act(nc.scalar.activation(ga[:, :, 0, :], gv[:, :, 0, :], AF.Tanh))
                c_act(
                    nc.scalar.activation(
                        ga[:, :, 1:3, :], gv[:, :, 1:3, :], AF.Sigmoid
                    )
                )
                c_act(
                    nc.scalar.activation(ga[:, :, 3, :], gv[:, :, 3, :], AF.Sigmoid)
                )
                t1 = tmp_pool.tile([128, 2, NT], DT.float32, tag="t1")
                t2 = tmp_pool.tile([128, 2, NT], DT.float32, tag="t2")
                c_dve(nc.vector.tensor_mul(t1[:], ga[:, :, 1, :], ga[:, :, 0, :]))
                c_dve(nc.vector.tensor_mul(t2[:], ga[:, :, 2, :], c_sb[:]))
                c_dve(nc.vector.tensor_add(c_sb[:], t1[:], t2[:]))
                th = tmp_pool.tile([128, 2, NT], DT.float32, tag="th")
                c_act(nc.scalar.activation(th[:], c_sb[:], AF.Tanh))

                if t < KL:
                    h16 = tmp_pool.tile([128, 2, NT], DT.float16, tag="h16")
                    c_dve(nc.vector.tensor_mul(h16[:], ga[:, :, 3, :], th[:]))
                    nc.sync.dma_start(out[t], h16[:])
                    fs = tmp_pool.tile([128, 2], DT.float32, tag="fs")
                    mult = tmp_pool.tile([128, 2], DT.float32, tag="mult")
                    c_dve(
                        nc.vector.tensor_reduce(
                            fs[:], ga[:, :, 2, B:NT], mybir.AxisListType.X, ALU.add
                        )
                    )
                    c_dve(nc.vector.tensor_scalar_mul(mult[:], fs[:], 1.0 / KB))
                    for s_ in range(2):
                        c_dve(
                            nc.vector.tensor_scalar_mul(
                                hstage[:, p, s_, 0:B],
                                h16[:, s_, 0:B],
                                mult[:, s_ : s_ + 1],
                            )
                        )
                        c_dve(
                            nc.vector.tensor_copy(
                                hstage[:, p, s_, B:NT], h16[:, s_, B:NT]
                            )
                        )
                        c_dve(
                            nc.vector.tensor_scalar_mul(
                                c_sb[:, s_, 0:B],
                                c_sb[:, s_, 0:B],
                                mult[:, s_ : s_ + 1],
                            )
                        )
                else:
                    c_dve(
                        nc.vector.tensor_mul(
                            hstage[:, p, :, :], ga[:, :, 3, :], th[:]
                        )
                    )

                # all-gather h via ncfw collective (HBM bounce); the ag_in
                # bounce DMA goes first - it is on the critical path and the
                # out DMA is not.
                ag_in = cdram_pool.tile([2 * 128, NT], DT.float16, tag="agin")
                nc.sync.dma_start(
                    ag_in.rearrange("(s p) b -> p s b", p=128), hstage[:, p, :, :]
                )
                if t >= KL:
                    nc.sync.dma_start(out[t], hstage[:, p, :, :])
                ag_out = cdram_pool.tile(
                    [H, NT], DT.float16, tag="agout", addr_space="Shared"
                )
                c_pl(
                    nc.gpsimd.collective_compute(
                        "AllGather",
                        ALU.bypass,
                        replica_groups=rg,
                        ins=[ag_in.opt()],
                        outs=[ag_out.opt()],
                    )
                )
                ag_re = ag_out.rearrange("(k p) b -> p k b", p=128)
                nc.sync.dma_start(h_sb[:, 0:4, :], ag_re[:, 0:4, :])
                nc.sync.dma_start(h_sb[:, 4:8, :], ag_re[:, 4:8, :])
                nc.scalar.dma_start(h_sb[:, 8:12, :], ag_re[:, 8:12, :])
                nc.scalar.dma_start(h_sb[:, 12:16, :], ag_re[:, 12:16, :])

    for binst, sem, val in deferred:
        binst.wait_op(sem, val, "sem-ge")

    nc.compile()
    return nc


def _prepare_inputs(x, key_seq, weight_ih, weight_hh, bias_ih, bias_hh, s_steps):
    x = np.ascontiguousarray(np.asarray(x, dtype=np.float32)[:, :s_steps, :])
    key_seq = np.asarray(key_seq, dtype=np.float32)
    weight_ih = np.asarray(weight_ih, dtype=np.float32)
    weight_hh = np.asarray(weight_hh, dtype=np.float32)
    b = np.asarray(bias_ih, dtype=np.float32) + np.asarray(bias_hh, dtype=np.float32)

    xt = np.ascontiguousarray(
        x.transpose(2, 1, 0).reshape(I, s_steps * B).astype(np.float16)
    )
    kt = np.ascontiguousarray(
        key_seq.transpose(2, 1, 0).reshape(I, KL * KB).astype(np.float16)
    )

    in_maps = []
    for j in range(NCORES):
        rows = _rows_for_core(j)
        in_maps.append(
            {
                "xt": xt,
                "kt": kt,
                "wih": np.ascontiguousarray(weight_ih[rows].T.astype(np.float16)),
                "whh": np.ascontiguousarray(weight_hh[rows].T.astype(np.float16)),
                "bias": np.ascontiguousarray(b[rows]),
            }
        )
    return in_maps


_NC_CACHE = {}


def _run(x, key_seq, weight_ih, weight_hh, bias_ih, bias_hh, s_steps, trace=False):
    if s_steps not in _NC_CACHE:
        _NC_CACHE[s_steps] = _build_program(s_steps)
    nc = _NC_CACHE[s_steps]
    in_maps = _prepare_inputs(
        x, key_seq, weight_ih, weight_hh, bias_ih, bias_hh, s_steps
    )
    res = bass_utils.run_bass_kernel_spmd(
        nc, in_maps, core_ids=list(range(NCORES)), trace=trace
    )
    full = np.empty((s_steps, B, H), dtype=np.float32)
    for j in range(NCORES):
        o = np.asarray(res.results[j]["out"], dtype=np.float32)  # [T,128,2,NT]
        # full[t, b, j*256 + s*128 + p] = o[t, p, s, b]
        full[:, :, j * HLOC : (j + 1) * HLOC] = (
            o[:, :, :, :B].transpose(0, 3, 2, 1).reshape(s_steps, B, HLOC)
        )
    return full, res


def kernel(x, key_seq, weight_ih, weight_hh, bias_ih, bias_hh):
    s_steps = int(os.environ.get("KEYED_LSTM_STEPS", S))
    trace = os.environ.get("KEYED_LSTM_TRACE", "0") == "1"
    full, _res = _run(
        x, key_seq, weight_ih, weight_hh, bias_ih, bias_hh, s_steps, trace=trace
    )
    return full
